# revision 11
# baseline (speedup 1.0000x reference)
"""ConformerAttention Trainium2 kernel (v3).

Math per batch b (biases are all zero in this problem):
  q = x @ (Wq/8); k = x @ Wk; v = x @ Wv
  scoresT[t, s] = k_t . q_s + pos[s, t]
  A = softmax over t (unnormalized exp; column sums Z via ones-matmul;
      1/Z folded into the AV PSUM->SBUF evacuation)
  Y = (AV/Z).T @ Wo

Layout strategy (per core: BC=256 batches, 32 blocks of NB=8):
  - x is transposed on HOST to xT [41, BC*41] bf16 and loaded to SBUF once.
  - Q/K projections head-PAIRED: one matmul per 128-wide u-chunk ->
    psum [128, W]; single [128, W] evacuation into paired SBUF tiles
    (even head rows 0:64, odd head rows 64:128).
  - V projection packs 3 batches per matmul (m=123).
  - Scores per pair: one pos-bias matmul (lhsT [41, 105] = [pos|0|pos],
    rhs = tiled one-hot, start=True zeroes the region), then per batch two
    plain k=64 matmuls: even head from rows 0:64 -> psum rows 0:41,
    odd head from rows 64:128 -> psum rows 64:105 (quadrant (64, 64)).
  - One exp per pair [105, W]; gpsimd relocates odd rows 64:105 to a
    base-0 tile so AV operands stay at partition base 0.
  - Z paired: lhsT [105, 128] ones pattern -> Z replicated over both
    64-row halves; reciprocal on vector; folded into AV evacuation.
  - Y: 3 row-groups x 2 halves x 8 accumulating matmuls (bf16), evacuated
    to a bf16 staging tile, one 3D-AP DMA store per block. Host upcasts.

Data parallel over batch: 8 cores x 256 batches.
"""

import math
import sys

import numpy as np

sys.path.insert(0, "/opt/trn_rl_repo")

import concourse.bass as bass
import concourse.bacc as bacc
import concourse.mybir as mybir
from concourse import tile
from concourse.bass_utils import run_bass_kernel_spmd

F32 = mybir.dt.float32
BF16 = mybir.dt.bfloat16

B, S, DIN = 2048, 41, 41
U, H, DH = 1024, 16, 64
NC = 8
BC = B // NC          # 256 batches per core
NB = 8                # batches per block
NBLK = BC // NB       # 32 blocks
W = NB * S            # 328 free-dim columns per block
NP = H // 2           # 8 head pairs == u-chunks of 128
YG = ((0, 110), (110, 110), (220, 108))  # Y row-groups within a block
VT = ((0, 3), (3, 6), (6, 8))            # V-projection batch triples


def build_kernel(nc: bass.Bass, nblk: int = NBLK):
    xt_d = nc.declare_dram_parameter("xt", [DIN, BC * S], BF16, isOutput=False)
    wq_d = nc.declare_dram_parameter("wq", [DIN, U], BF16, isOutput=False)
    wk_d = nc.declare_dram_parameter("wk", [DIN, U], BF16, isOutput=False)
    wv_d = nc.declare_dram_parameter("wv", [DIN, U], BF16, isOutput=False)
    wo_d = nc.declare_dram_parameter("wo", [U, U], BF16, isOutput=False)
    pos_d = nc.declare_dram_parameter("pos105", [S, 105], BF16, isOutput=False)
    oh_d = nc.declare_dram_parameter("oh_pat", [S, W], BF16, isOutput=False)
    ones_d = nc.declare_dram_parameter("ones105", [105, 128], BF16, isOutput=False)
    out_d = nc.declare_dram_parameter("out", [BC * S, U], BF16, isOutput=True)

    from contextlib import ExitStack
    with tile.TileContext(nc) as tc, ExitStack() as st:
        cpool = st.enter_context(tc.tile_pool(name="consts", bufs=1))
        qkp = st.enter_context(tc.tile_pool(name="qk", bufs=2))
        esp = st.enter_context(tc.tile_pool(name="es", bufs=10))
        vp = st.enter_context(tc.tile_pool(name="v", bufs=16))
        otnp = st.enter_context(tc.tile_pool(name="otn", bufs=10))
        rzp = st.enter_context(tc.tile_pool(name="rz", bufs=3))
        yp = st.enter_context(tc.tile_pool(name="y", bufs=2))
        ps = st.enter_context(tc.tile_pool(name="ps", bufs=1, space="PSUM"))

        # ---- constants ----
        wq_s = cpool.tile([DIN, U], BF16, tag="wq")
        wk_s = cpool.tile([DIN, U], BF16, tag="wk")
        wv_s = cpool.tile([DIN, U], BF16, tag="wv")
        nc.sync.dma_start(wq_s[:], wq_d[:])
        nc.sync.dma_start(wk_s[:], wk_d[:])
        nc.sync.dma_start(wv_s[:], wv_d[:])
        wo_s = []
        for c in range(NP):
            t = cpool.tile([128, U], BF16, tag=f"wo{c}")
            nc.sync.dma_start(t[:], wo_d[c * 128:(c + 1) * 128, :])
            wo_s.append(t)
        pos_s = cpool.tile([S, 105], BF16, tag="pos")
        nc.sync.dma_start(pos_s[:], pos_d[:])
        oh_s = cpool.tile([S, W], BF16, tag="oh")
        nc.sync.dma_start(oh_s[:], oh_d[:])
        ones_s = cpool.tile([105, 128], BF16, tag="ones")
        nc.sync.dma_start(ones_s[:], ones_d[:])
        xt_all = cpool.tile([DIN, BC * S], BF16, tag="xt")
        nc.sync.dma_start(xt_all[:], xt_d[:])

        for blk in range(nblk):
            col0 = blk * W

            # ---- Q/K projections, head-paired ----
            qp, kp = [], []
            for c in range(NP):
                qps = ps.tile([128, W], F32, tag="qk", bufs=2)
                nc.tensor.matmul(qps[:], wq_s[:, c * 128:(c + 1) * 128],
                                 xt_all[:, col0:col0 + W])
                qt = qkp.tile([128, W], BF16, tag="qp")
                nc.vector.tensor_copy(qt[:], qps[:])
                qp.append(qt)
                kps = ps.tile([128, W], F32, tag="qk", bufs=2)
                nc.tensor.matmul(kps[:], wk_s[:, c * 128:(c + 1) * 128],
                                 xt_all[:, col0:col0 + W])
                kt = qkp.tile([128, W], BF16, tag="kp")
                nc.vector.tensor_copy(kt[:], kps[:])
                kp.append(kt)

            # ---- V projection: 2 batches per psum tile (rows 0:41 / 64:105) ----
            vt = [vp.tile([S, U], BF16, tag="v", name=f"v{b}") for b in range(NB)]
            for bp in range(NB // 2):
                b0, b1 = 2 * bp, 2 * bp + 1
                for half in range(2):
                    vps = ps.tile([128, 512], F32, tag="vy", bufs=2)
                    hs = slice(half * 512, (half + 1) * 512)
                    nc.tensor.matmul(vps[0:S, :],
                                     xt_all[:, col0 + b0 * S:col0 + (b0 + 1) * S],
                                     wv_s[:, hs], skip_group_check=True)
                    nc.tensor.matmul(vps[64:64 + S, :],
                                     xt_all[:, col0 + b1 * S:col0 + (b1 + 1) * S],
                                     wv_s[:, hs], skip_group_check=True)
                    nc.vector.tensor_copy(vt[b0][:, hs], vps[0:S, :])
                    nc.vector.tensor_copy(vt[b1][:, hs], vps[64:64 + S, :])

            # ---- scores + pos bias + exp, one psum tile per head pair ----
            es, eso = [], []
            for c in range(NP):
                sp = ps.tile([105, W], F32, tag="s", bufs=2)
                nc.tensor.matmul(sp[:], pos_s[:], oh_s[:],
                                 start=True, stop=False, skip_group_check=True)
                for b in range(NB):
                    cs = slice(b * S, (b + 1) * S)
                    nc.tensor.matmul(sp[0:S, cs], kp[c][0:DH, cs], qp[c][0:DH, cs],
                                     start=False, stop=False, skip_group_check=True)
                    nc.tensor.matmul(sp[64:64 + S, cs], kp[c][DH:128, cs],
                                     qp[c][DH:128, cs],
                                     start=False, stop=(b == NB - 1),
                                     skip_group_check=True)
                e = esp.tile([105, W], BF16, tag="es")
                nc.scalar.activation(e[:], sp[:], mybir.ActivationFunctionType.Exp)
                es.append(e)
                eo = esp.tile([S, W], BF16, tag="eso")
                nc.sync.dma_start(eo[:], e[64:64 + S, :])
                eso.append(eo)

            # ---- per u-chunk: Z (paired), reciprocal, AV, normalize ----
            otn = []
            for c in range(NP):
                zps = ps.tile([128, W], F32, tag="z", bufs=1)
                nc.tensor.matmul(zps[:], ones_s[:], es[c][:])
                rz = rzp.tile([128, W], F32, tag="rz")
                nc.vector.reciprocal_approx_fast(rz[:], zps[:])

                ops_ = ps.tile([128, W], F32, tag="av", bufs=1)
                for b in range(NB):
                    cs = slice(b * S, (b + 1) * S)
                    nc.tensor.matmul(ops_[0:DH, cs],
                                     vt[b][:, (2 * c) * DH:(2 * c + 1) * DH],
                                     es[c][0:S, cs])
                    nc.tensor.matmul(ops_[DH:128, cs],
                                     vt[b][:, (2 * c + 1) * DH:(2 * c + 2) * DH],
                                     eso[c][:, cs])
                on = otnp.tile([128, W], BF16, tag="otn")
                nc.vector.tensor_mul(on[:], ops_[:], rz[:])
                otn.append(on)

            # ---- output projection Y ----
            y = yp.tile([128, 3 * U], BF16, tag="y")
            for g, (r0, rows) in enumerate(YG):
                for half in range(2):
                    yps = ps.tile([128, 512], F32, tag="vy", bufs=2)
                    for c in range(NP):
                        nc.tensor.matmul(
                            yps[:rows, :],
                            otn[c][:, r0:r0 + rows],
                            wo_s[c][:, half * 512:(half + 1) * 512],
                            start=(c == 0), stop=(c == NP - 1))
                    dst = y[:rows, g * U + half * 512:g * U + (half + 1) * 512]
                    if half == 0:
                        nc.scalar.copy(dst, yps[:rows, :])
                    else:
                        nc.vector.tensor_copy(dst, yps[:rows, :])
            for g, (r0, rows) in enumerate(YG):
                nc.sync.dma_start(out_d[blk * W + r0:blk * W + r0 + rows, :],
                                  y[0:rows, g * U:(g + 1) * U])

    return nc


_NC_CACHE = {}


def get_nc():
    if "nc" not in _NC_CACHE:
        nc = bacc.Bacc(None, target_bir_lowering=False)
        build_kernel(nc)
        nc.compile()
        _NC_CACHE["nc"] = nc
    return _NC_CACHE["nc"]


def host_inputs(x, Wq, Wk, Wv, Wo, rel_emb):
    """Prepare the per-core DRAM input maps (layout/precast on host)."""
    import ml_dtypes
    bf = lambda a: np.ascontiguousarray(np.asarray(a, np.float32)).astype(ml_dtypes.bfloat16)

    idx = np.clip(np.arange(-20, 21), -S + 1, S - 1) + 20
    pos = np.asarray(rel_emb, np.float32)[idx]              # (41, 41): pos[s, t]
    pos105 = np.zeros((S, 105), np.float32)
    pos105[:, 0:S] = pos
    pos105[:, 64:64 + S] = pos
    oh_pat = np.tile(np.eye(S, dtype=np.float32), (1, NB))  # (41, 328)
    ones105 = np.zeros((105, 128), np.float32)
    ones105[0:S, 0:DH] = 1.0
    ones105[64:64 + S, DH:128] = 1.0
    wq_scaled = np.asarray(Wq, np.float32) / math.sqrt(DH)

    m0 = {
        "wq": bf(wq_scaled), "wk": bf(Wk), "wv": bf(Wv), "wo": bf(Wo),
        "pos105": bf(pos105), "oh_pat": bf(oh_pat), "ones105": bf(ones105),
    }
    x = np.asarray(x, np.float32)
    in_maps = []
    for ci in range(NC):
        xi = x[ci * BC:(ci + 1) * BC].reshape(BC * S, DIN).T  # (41, BC*S)
        in_maps.append({**m0, "xt": bf(xi)})
    return in_maps


def kernel(x, Wq, bq, Wk, bk, Wv, bv, Wo, bo, rel_emb):
    in_maps = host_inputs(x, Wq, Wk, Wv, Wo, rel_emb)
    nc = get_nc()
    res = run_bass_kernel_spmd(nc, in_maps, core_ids=list(range(NC)))
    out = np.concatenate(
        [np.asarray(res.results[i]["out"], np.float32).reshape(BC, S, U)
         for i in range(NC)], axis=0)
    return out
